# revision 39
# baseline (speedup 1.0000x reference)
"""Trainium2 Bass kernel for causal self-attention (B=2, T=2048, C=1024, H=16).

Sharding: tensor-parallel over heads x data-parallel over batch.
Each of the 8 cores handles one (batch b, head-group g) pair: b = core // 4,
g = core % 4, where a head group is 4 consecutive heads (heads 4g..4g+3).

v2 pipeline (single pass, both head-pairs interleaved by q-chunk):
  - prelude: qkv chains for q-chunk 0 emitted chunk-major across all 8 PSUM
    banks so they pipeline with the xT input DMA.
  - per qc: attention(pair0, qc) then attention(pair1, qc); qk/v chains for
    qc+1 and output-projection chunks for qc-1 drip into the attention steps
    as PE fillers, so the exp stream on ACT never starves and the PE never
    head-of-line blocks.
  - attention step = 2 k-blocks x 2 heads: S^T matmuls (the two heads run in
    different PE row groups, concurrently), exp on ACT from PSUM, causal mask
    multiply on DVE, and the PV accumulation lagged by one step so it never
    stalls the PE on the exp/mask latency.
  - softmax denominator rides as a 65th ones-column in v (row 64 of the PV
    accumulator); normalize = DVE copy -> gpsimd partition_broadcast ->
    DVE reciprocal+multiply (nothing on ACT except exp).
  - output partials are written bf16 (the host all-reduce sums them in f32).
"""

import numpy as np
from collections import deque
from contextlib import ExitStack

import concourse.bass as bass
import concourse.tile as tile
from concourse import bacc, library_config, mybir
from concourse.bass import ts
from concourse.bass_utils import run_bass_kernel_spmd

F32 = mybir.dt.float32
BF16 = mybir.dt.bfloat16
AF = mybir.ActivationFunctionType
PSUM = bass.MemorySpace.PSUM

B, T, C, H = 2, 2048, 1024, 16
HD = C // H              # 64
HPC = 4                  # heads per core
PAIRS = 2                # head pairs per core
CI = C // 128            # 8 contraction chunks
TB = T // 128            # 16 t-blocks
NQC = T // 512           # 4 q-chunks
N_CORES = 8

DRIP_N = 3               # filler advances per attention step


def _emit(tc, nc, xT_d, w0_d, wv_d, w1_d, wp_d, out_d):
    ctx = ExitStack()
    with ctx:
        pers = ctx.enter_context(tc.tile_pool(name="pers", bufs=1))
        nc.gpsimd.load_library(library_config.attn)

        qT = [pers.tile([128, T], BF16, name=f"qT{p}") for p in range(PAIRS)]
        kT = [pers.tile([128, T], BF16, name=f"kT{p}") for p in range(PAIRS)]
        # v storage: [128, tb, head, 65]; column 64 of each (tb, head) block
        # is 1.0 so the PV matmul's row 64 accumulates the softmax denominator
        v_all = pers.tile([128, TB, HPC, 65], BF16, name="v_all")
        yT = [pers.tile([128, T], BF16, name=f"yT{p}") for p in range(PAIRS)]
        # w0: [wk0 | wq0], w1: [wk1 | wq1] — packed so each lands in one DMA
        w0_sb = pers.tile([128, 2048], BF16, name="w0")
        wv_sb = pers.tile([128, 2048], BF16, name="wv")
        w1_sb = pers.tile([128, 2048], BF16, name="w1")
        wp_sb = pers.tile([128, 2048], BF16, name="wp")
        xT_tiles = [pers.tile([128, 2 * T], BF16, name=f"xt{i}")
                    for i in range(3)] + [
                    pers.tile([128, T], BF16, name=f"xs{i}") for i in range(2)]
        mask_d = pers.tile([128, 128], BF16, name="mask_d")
        ones_f = pers.tile([128, 1], F32, name="ones_f")

        # input DMAs: weights dispatch on the scalar HWDGE queue, xT on the
        # sync queue, so dispatch overlaps and the prelude's v-chain matmuls
        # never head-of-line block the PE waiting for wv
        nc.scalar.dma_start(w0_sb[:], w0_d[:])
        nc.scalar.dma_start(wv_sb[:], wv_d[:])
        for i in range(3):
            nc.sync.dma_start(
                xT_tiles[i][:].rearrange("p (c t) -> p c t", c=2),
                xT_d[2 * i:2 * i + 2, :, :].rearrange("c p t -> p c t"),
            )
        for i in range(2):
            nc.sync.dma_start(xT_tiles[3 + i][:], xT_d[6 + i])
        nc.sync.dma_start(w1_sb[:], w1_d[:])
        nc.scalar.dma_start(wp_sb[:], wp_d[:])
        xT_sb = [xT_tiles[ci // 2][:, (ci % 2) * T:(ci % 2) * T + T]
                 for ci in range(6)] + [xT_tiles[3][:], xT_tiles[4][:]]
        wk_sb = [w0_sb[:, 0:1024], w1_sb[:, 0:1024]]
        wq_sb = [w0_sb[:, 1024:2048], w1_sb[:, 1024:2048]]

        # constants
        nc.gpsimd.memset(ones_f[:], 1.0)
        for h in range(HPC):
            nc.vector.tensor_copy(
                v_all[:, :, h, 64:65],
                ones_f[:].unsqueeze(1).broadcast_to([128, TB, 1]),
            )
        mask_f = pers.tile([128, 128], F32, name="mask_f")
        nc.gpsimd.memset(mask_f[:], 1.0)
        nc.gpsimd.affine_select(
            out=mask_f[:], in_=mask_f[:],
            compare_op=mybir.AluOpType.is_ge, fill=0.0,
            base=0, channel_multiplier=-1, pattern=[[1, 128]],
        )
        nc.vector.tensor_copy(mask_d[:], mask_f[:])

        # ---- prelude: qc0 chains for pair0 + v t-blocks 0..3, chunk-major
        # across 6 psum banks so they pipeline with the xT DMAs ----
        with tc.tile_pool(name="psPre", bufs=1, space=PSUM) as psPre:
            pre_qk = [(wk_sb[0], kT[0]), (wq_sb[0], qT[0])]
            qk_ps = [psPre.tile([128, 512], F32, tag=f"pre{i}", name=f"pre{i}")
                     for i in range(len(pre_qk))]
            v_ps = [psPre.tile([128, 256], F32, tag=f"prev{tb}", name=f"prev{tb}")
                    for tb in range(4)]
            for ci in range(CI):
                for i, (w_sb, _) in enumerate(pre_qk):
                    nc.tensor.matmul(
                        qk_ps[i][:], w_sb[:, ts(ci, 128)], xT_sb[ci][:, 0:512],
                        start=(ci == 0), stop=(ci == CI - 1),
                    )
                for tb in range(4):
                    nc.tensor.matmul(
                        v_ps[tb][:], xT_sb[ci][:, ts(tb, 128)],
                        wv_sb[:, ts(ci, 256)],
                        start=(ci == 0), stop=(ci == CI - 1),
                    )
            for i, (_, dst) in enumerate(pre_qk):
                nc.vector.tensor_copy(dst[:, 0:512], qk_ps[i][:])
            for tb in range(4):
                nc.vector.tensor_copy(
                    v_all[:, tb, :, 0:64],
                    v_ps[tb][:].rearrange("p (h d) -> p h d", d=HD),
                )

        with (
            tc.tile_pool(name="psS", bufs=1, space=PSUM) as psS,
            tc.tile_pool(name="psY", bufs=1, space=PSUM) as psY,
            tc.tile_pool(name="pF", bufs=2, space=PSUM) as pF,
            tc.tile_pool(name="pP", bufs=4) as pP,
            tc.tile_pool(name="pN", bufs=3) as pN,
            tc.tile_pool(name="pO", bufs=3) as pO,
        ):
            # ---- filler generators (each yield ~= 2 matmuls of PE work) ----
            def gen_qk(w_sb, dst, qc):
                ps = pF.tile([128, 512], F32, tag="fill", name="fqk")
                for ci in range(CI):
                    nc.tensor.matmul(
                        ps[:], w_sb[:, ts(ci, 128)], xT_sb[ci][:, ts(qc, 512)],
                        start=(ci == 0), stop=(ci == CI - 1),
                    )
                    if ci % 2 == 1 and ci < CI - 1:
                        yield
                nc.vector.tensor_copy(dst[:, ts(qc, 512)], ps[:])

            def gen_v(tb):
                psv = pF.tile([128, 256], F32, tag="fill", name="fv")
                for ci in range(CI):
                    nc.tensor.matmul(
                        psv[:], xT_sb[ci][:, ts(tb, 128)],
                        wv_sb[:, ts(ci, 256)],
                        start=(ci == 0), stop=(ci == CI - 1),
                    )
                    if ci % 2 == 1 and ci < CI - 1:
                        yield
                nc.vector.tensor_copy(
                    v_all[:, tb, :, 0:64],
                    psv[:].rearrange("p (h d) -> p h d", d=HD),
                )

            def gen_proj(tb, scalar_cast=False):
                ob = pO.tile([128, 1024], BF16, tag="ob", name=f"ob{tb}")
                for cc in range(2):
                    po = pF.tile([128, 512], F32, tag="fill", name="fpo")
                    for p in range(PAIRS):
                        nc.tensor.matmul(
                            po[:], yT[p][:, ts(tb, 128)],
                            wp_sb[:, p * 1024 + cc * 512:
                                  p * 1024 + cc * 512 + 512],
                            start=(p == 0), stop=(p == PAIRS - 1),
                        )
                    if scalar_cast:   # tail units: ACT is idle by then
                        nc.scalar.copy(ob[:, ts(cc, 512)], po[:])
                    else:
                        nc.vector.tensor_copy(ob[:, ts(cc, 512)], po[:])
                    yield
                nc.sync.dma_start(out_d[ts(tb, 128), :], ob[:])

            fillers = deque()        # urgent: deadline-bound qk/v chains
            fillers_lazy = deque()   # lazy: proj chunks (no deadline)
            reserve = []             # proj units held back for the tail
            active = [None]
            done = set()

            def _advance(no_lazy=False):
                """Advance the head filler generator one yield; True if work done."""
                while True:
                    if active[0] is None:
                        if fillers:
                            active[0] = fillers.popleft()
                        elif fillers_lazy and not no_lazy:
                            active[0] = fillers_lazy.popleft()
                        else:
                            return False
                    try:
                        next(active[0][1])
                        return True
                    except StopIteration:
                        done.add(active[0][0])
                        active[0] = None

            def drip(n=DRIP_N, no_lazy=False):
                for _ in range(n):
                    if not _advance(no_lazy):
                        return

            def drain(label):
                """Force-finish all fillers up to and including `label`."""
                while label not in done:
                    if not _advance():
                        return

            # ---- attention ----
            def attn_qc(p, qc):
                nkb = 4 * qc + 4
                ypt = [psY.tile([128, 512], F32, tag=f"ypt{hh}", bufs=1,
                                name=f"ypt_p{p}q{qc}h{hh}") for hh in (0, 1)]

                def emit_pv(kb0, pts):
                    for j in (0, 1):
                        kb = kb0 + j
                        col = max(0, (kb - 4 * qc) * 128)
                        for hh in (0, 1):
                            nc.tensor.matmul(
                                ypt[hh][0:65, col:512],
                                v_all[:, kb, 2 * p + hh, 0:65],
                                pts[hh][:, j * 512 + col: (j + 1) * 512],
                                start=(kb == 0), stop=(kb == nkb - 1),
                            )

                pend = None
                for kb0 in range(0, nkb, 2):
                    if p == 0 and qc > 0 and kb0 == 4 * qc:
                        drain(f"v{4 * qc + 3}")   # diag PVs need this qc's v
                    sps = [psS.tile([128, 1024], F32, tag=f"sp{hh}", bufs=1,
                                    name=f"sp{hh}") for hh in (0, 1)]
                    for hh in (0, 1):
                        off = hh * 64
                        for j in (0, 1):
                            kb = kb0 + j
                            col = max(0, (kb - 4 * qc) * 128)
                            nc.tensor.matmul(
                                sps[hh][:, j * 512 + col: (j + 1) * 512],
                                kT[p][off:off + 64, ts(kb, 128)],
                                qT[p][off:off + 64,
                                      qc * 512 + col: (qc + 1) * 512],
                                start=True, stop=True,
                            )
                    pts = []
                    for hh in (0, 1):
                        pt = pP.tile([128, 1024], BF16, tag=f"pt{hh}",
                                     name=f"pt{hh}")
                        nc.scalar.activation(pt[:], sps[hh][:], AF.Exp)
                        for j in (0, 1):
                            kb = kb0 + j
                            if kb >= 4 * qc:
                                col = j * 512 + (kb - 4 * qc) * 128
                                nc.vector.tensor_mul(
                                    pt[:, col:col + 128],
                                    pt[:, col:col + 128], mask_d[:],
                                )
                        pts.append(pt)
                    drip()
                    if pend is not None:
                        emit_pv(*pend)
                    pend = (kb0, pts)
                emit_pv(*pend)
                # normalize: yT = num * (1/l); l lives in row 64 of ypt.
                # In the final pass the l-copy runs on ACT (idle by then) to
                # shorten the latency chain gating the last projections.
                for hh in (0, 1):
                    off = hh * 64
                    l_sb = pN.tile([1, 512], F32, tag="lr")
                    if p == 1 and qc == NQC - 1:
                        nc.scalar.copy(l_sb[:], ypt[hh][64:65, :])
                    else:
                        nc.vector.tensor_copy(l_sb[:], ypt[hh][64:65, :])
                    il = pN.tile([1, 512], F32, tag="il")
                    nc.vector.reciprocal_approx_fast(il[:], l_sb[:])
                    rl = pN.tile([64, 512], F32, tag="rl")
                    nc.gpsimd.partition_broadcast(rl[:], il[:])
                    nc.vector.tensor_mul(
                        yT[p][off:off + 64, ts(qc, 512)], ypt[hh][0:64, :],
                        rl[:],
                    )

            # ---- main schedule ----
            # queue order per qc: chains for qc+1 (deadline-bound, drained
            # just-in-time) then proj for qc (deadline-free; leftovers keep
            # the PE warm through the final normalize + projection tail)
            # supply shape: k0/q0 chains for qc+1 drip through iteration qc
            # (they gate the next pass); v + pair-1 chains for qc drip inside
            # iteration qc itself so the long late passes keep the PE fed
            fillers.append(("k1q0", gen_qk(wk_sb[1], kT[1], 0)))
            fillers.append(("q1q0", gen_qk(wq_sb[1], qT[1], 0)))
            for qc in range(NQC):
                if qc > 0:
                    for tb in range(4 * qc, 4 * qc + 4):
                        fillers.append((f"v{tb}", gen_v(tb)))
                    fillers.append((f"k1q{qc}", gen_qk(wk_sb[1], kT[1], qc)))
                    fillers.append((f"q1q{qc}", gen_qk(wq_sb[1], qT[1], qc)))
                if qc + 1 < NQC:
                    nqc = qc + 1
                    fillers.append((f"k0q{nqc}", gen_qk(wk_sb[0], kT[0], nqc)))
                    fillers.append((f"q0q{nqc}", gen_qk(wq_sb[0], qT[0], nqc)))
                attn_qc(0, qc)
                drain(f"q1q{qc}")
                if qc + 1 < NQC:
                    # force next-pass chains now: the burst overlaps this
                    # qc's pair-1 exp stream instead of the pass boundary
                    drain(f"q0q{qc + 1}")
                attn_qc(1, qc)
                for tb in range(4 * qc, 4 * qc + 4):
                    # LIFO for qc<3 so late passes drip the newest proj first;
                    # two qc2 units are held back to cover the final
                    # normalize window; qc3's own (normalize-gated) units
                    # stay at the back so the tail never stalls on them
                    if qc == NQC - 1:
                        fillers_lazy.append((f"proj{tb}", gen_proj(tb, True)))
                    elif qc == NQC - 2 and tb >= 4 * qc + 2:
                        reserve.append((f"proj{tb}", gen_proj(tb)))
                    else:
                        fillers_lazy.appendleft((f"proj{tb}", gen_proj(tb)))
            fillers_lazy.extendleft(reversed(reserve))
            while _advance():
                pass


_NC_CACHE = None


def _build():
    global _NC_CACHE
    if _NC_CACHE is not None:
        return _NC_CACHE
    nc = bacc.Bacc("TRN2", target_bir_lowering=False, debug=False,
                   num_devices=N_CORES)
    xT_d = nc.dram_tensor("xT", [CI, 128, T], BF16, kind="ExternalInput")
    w0_d = nc.dram_tensor("w0", [128, 2048], BF16, kind="ExternalInput")
    wv_d = nc.dram_tensor("wv", [128, 2048], BF16, kind="ExternalInput")
    w1_d = nc.dram_tensor("w1", [128, 2048], BF16, kind="ExternalInput")
    wp_d = nc.dram_tensor("wp", [128, 2048], BF16, kind="ExternalInput")
    out_d = nc.dram_tensor("out", [T, C], BF16, kind="ExternalOutput")

    with tile.TileContext(nc) as tc:
        _emit(tc, nc, xT_d, w0_d, wv_d, w1_d, wp_d, out_d)
    nc.compile()
    _NC_CACHE = nc
    return nc


def _pack_pair(m):
    # [1024, 128] -> lhsT chunks layout [128, 8*128]
    return np.ascontiguousarray(
        m.reshape(CI, 128, 128).transpose(1, 0, 2).reshape(128, 1024))


def _io_np(a):
    import ml_dtypes
    return np.ascontiguousarray(a.astype(ml_dtypes.bfloat16))


def _in_maps(x, w_attn, w_proj):
    x = np.asarray(x, dtype=np.float32)
    w_attn = np.asarray(w_attn, dtype=np.float32)
    w_proj = np.asarray(w_proj, dtype=np.float32)
    xT = [_io_np(np.ascontiguousarray(x[b].T).reshape(CI, 128, T))
          for b in range(B)]
    maps = []
    for core in range(N_CORES):
        b, g = core // HPC, core % HPC
        cols = slice(g * 256, (g + 1) * 256)
        wk_full = w_attn[:, 0 * C:1 * C][:, cols]
        wq_full = w_attn[:, 1 * C:2 * C][:, cols] * np.float32(1.0 / np.sqrt(HD))
        wv_full = w_attn[:, 2 * C:3 * C][:, cols]
        wq_in = [_pack_pair(wq_full[:, p * 128:(p + 1) * 128])
                 for p in range(PAIRS)]
        wk_in = [_pack_pair(wk_full[:, p * 128:(p + 1) * 128])
                 for p in range(PAIRS)]
        wv_in = wv_full.reshape(CI, 128, 256).transpose(1, 0, 2).reshape(128, 2048)
        wp_in = (w_proj[g * 256:(g + 1) * 256, :]
                 .reshape(PAIRS, 128, 1024).transpose(1, 0, 2).reshape(128, 2048))
        w0_in = np.concatenate([wk_in[0], wq_in[0]], axis=1)
        w1_in = np.concatenate([wk_in[1], wq_in[1]], axis=1)
        maps.append({"xT": xT[b], "w0": _io_np(w0_in), "wv": _io_np(wv_in),
                     "w1": _io_np(w1_in), "wp": _io_np(wp_in)})
    return maps


def _assemble(results, b_proj):
    b_proj = np.asarray(b_proj, dtype=np.float32)
    out = np.zeros((B, T, C), dtype=np.float32)
    for core in range(N_CORES):
        out[core // HPC] += np.asarray(results[core]["out"], dtype=np.float32)
    out += b_proj[None, None, :]
    return out


def kernel(x, w_attn, w_proj, b_proj):
    nc = _build()
    maps = _in_maps(x, w_attn, w_proj)
    res = run_bass_kernel_spmd(nc, maps, list(range(N_CORES)))
    return _assemble(res.results, b_proj)


def kernel_traced(x, w_attn, w_proj, b_proj):
    """Like kernel() but with NTFF tracing; returns (out, BassKernelResults)."""
    nc = _build()
    maps = _in_maps(x, w_attn, w_proj)
    res = run_bass_kernel_spmd(nc, maps, list(range(N_CORES)), trace=True)
    return _assemble(res.results, b_proj), res


# revision 52
# speedup vs baseline: 1.0570x; 1.0570x over previous
"""Trainium2 Bass kernel for causal self-attention (B=2, T=2048, C=1024, H=16).

Sharding: tensor-parallel over heads x data-parallel over batch.
Each of the 8 cores handles one (batch b, head-group g) pair: b = core // 4,
g = core % 4, where a head group is 4 consecutive heads (heads 4g..4g+3).

v2 pipeline (single pass, both head-pairs interleaved by q-chunk):
  - prelude: qkv chains for q-chunk 0 emitted chunk-major across all 8 PSUM
    banks so they pipeline with the xT input DMA.
  - per qc: attention(pair0, qc) then attention(pair1, qc); qk/v chains for
    qc+1 and output-projection chunks for qc-1 drip into the attention steps
    as PE fillers, so the exp stream on ACT never starves and the PE never
    head-of-line blocks.
  - attention step = 2 k-blocks x 2 heads: S^T matmuls (the two heads run in
    different PE row groups, concurrently), exp on ACT from PSUM, causal mask
    multiply on DVE, and the PV accumulation lagged by one step so it never
    stalls the PE on the exp/mask latency.
  - softmax denominator rides as a 65th ones-column in v (row 64 of the PV
    accumulator); normalize = DVE copy -> gpsimd partition_broadcast ->
    DVE reciprocal+multiply (nothing on ACT except exp).
  - output partials are written bf16 (the host all-reduce sums them in f32).
"""

import numpy as np
from collections import deque
from contextlib import ExitStack

import concourse.bass as bass
import concourse.tile as tile
from concourse import bacc, library_config, mybir
from concourse.bass import ts
from concourse.bass_utils import run_bass_kernel_spmd

F32 = mybir.dt.float32
BF16 = mybir.dt.bfloat16
AF = mybir.ActivationFunctionType
PSUM = bass.MemorySpace.PSUM

B, T, C, H = 2, 2048, 1024, 16
HD = C // H              # 64
HPC = 4                  # heads per core
PAIRS = 2                # head pairs per core
CI = C // 128            # 8 contraction chunks
TB = T // 128            # 16 t-blocks
NQC = T // 512           # 4 q-chunks
N_CORES = 8

DRIP_N = 3               # filler advances per attention step


def _emit(tc, nc, xT_d, w0_d, wv_d, w1_d, wp_d, out_d):
    ctx = ExitStack()
    with ctx:
        pers = ctx.enter_context(tc.tile_pool(name="pers", bufs=1))
        nc.gpsimd.load_library(library_config.attn)

        qT = [pers.tile([128, T], BF16, name=f"qT{p}") for p in range(PAIRS)]
        kT = [pers.tile([128, T], BF16, name=f"kT{p}") for p in range(PAIRS)]
        # v storage: [128, tb, head, 65]; column 64 of each (tb, head) block
        # is 1.0 so the PV matmul's row 64 accumulates the softmax denominator
        v_all = pers.tile([128, TB, HPC, 65], BF16, name="v_all")
        yT = [pers.tile([128, T], BF16, name=f"yT{p}") for p in range(PAIRS)]
        # w0: [wk0 | wq0], w1: [wk1 | wq1] — packed so each lands in one DMA
        w0_sb = pers.tile([128, 2048], BF16, name="w0")
        wv_sb = pers.tile([128, 2048], BF16, name="wv")
        w1_sb = pers.tile([128, 2048], BF16, name="w1")
        wp_sb = pers.tile([128, 2048], BF16, name="wp")
        xT_tiles = [pers.tile([128, 2 * T], BF16, name=f"xt{i}")
                    for i in range(3)] + [
                    pers.tile([128, T], BF16, name=f"xs{i}") for i in range(2)]
        mask_d = pers.tile([128, 128], BF16, name="mask_d")
        ones_f = pers.tile([128, 1], F32, name="ones_f")

        # input DMAs: weights dispatch on the scalar HWDGE queue, xT on the
        # sync queue, so dispatch overlaps and the prelude's v-chain matmuls
        # never head-of-line block the PE waiting for wv
        nc.scalar.dma_start(w0_sb[:], w0_d[:])
        nc.scalar.dma_start(wv_sb[:], wv_d[:])
        for i in range(3):
            nc.sync.dma_start(
                xT_tiles[i][:].rearrange("p (c t) -> p c t", c=2),
                xT_d[2 * i:2 * i + 2, :, :].rearrange("c p t -> p c t"),
            )
        for i in range(2):
            nc.sync.dma_start(xT_tiles[3 + i][:], xT_d[6 + i])
        nc.sync.dma_start(w1_sb[:], w1_d[:])
        nc.scalar.dma_start(wp_sb[:], wp_d[:])
        xT_sb = [xT_tiles[ci // 2][:, (ci % 2) * T:(ci % 2) * T + T]
                 for ci in range(6)] + [xT_tiles[3][:], xT_tiles[4][:]]
        wk_sb = [w0_sb[:, 0:1024], w1_sb[:, 0:1024]]
        wq_sb = [w0_sb[:, 1024:2048], w1_sb[:, 1024:2048]]

        # constants
        nc.gpsimd.memset(ones_f[:], 1.0)
        for h in range(HPC):
            nc.vector.tensor_copy(
                v_all[:, :, h, 64:65],
                ones_f[:].unsqueeze(1).broadcast_to([128, TB, 1]),
            )
        mask_f = pers.tile([128, 128], F32, name="mask_f")
        nc.gpsimd.memset(mask_f[:], 1.0)
        nc.gpsimd.affine_select(
            out=mask_f[:], in_=mask_f[:],
            compare_op=mybir.AluOpType.is_ge, fill=0.0,
            base=0, channel_multiplier=-1, pattern=[[1, 128]],
        )
        nc.vector.tensor_copy(mask_d[:], mask_f[:])

        # ---- prelude: qc0 chains for pair0 + v t-blocks 0..3, chunk-major
        # across 6 psum banks so they pipeline with the xT DMAs ----
        with tc.tile_pool(name="psPre", bufs=1, space=PSUM) as psPre:
            pre_qk = [(wk_sb[0], kT[0]), (wq_sb[0], qT[0]),
                      (wk_sb[1], kT[1]), (wq_sb[1], qT[1])]
            qk_ps = [psPre.tile([128, 512], F32, tag=f"pre{i}", name=f"pre{i}")
                     for i in range(len(pre_qk))]
            v_ps = [psPre.tile([128, 256], F32, tag=f"prev{tb}", name=f"prev{tb}")
                    for tb in range(4)]
            for ci in range(CI):
                for i, (w_sb, _) in enumerate(pre_qk[:2]):
                    nc.tensor.matmul(
                        qk_ps[i][:], w_sb[:, ts(ci, 128)], xT_sb[ci][:, 0:512],
                        start=(ci == 0), stop=(ci == CI - 1),
                    )
                for tb in range(4):
                    nc.tensor.matmul(
                        v_ps[tb][:], xT_sb[ci][:, ts(tb, 128)],
                        wv_sb[:, ts(ci, 256)],
                        start=(ci == 0), stop=(ci == CI - 1),
                    )
                for i, (w_sb, _) in enumerate(pre_qk[2:], start=2):
                    nc.tensor.matmul(
                        qk_ps[i][:], w_sb[:, ts(ci, 128)], xT_sb[ci][:, 0:512],
                        start=(ci == 0), stop=(ci == CI - 1),
                    )
            for i, (_, dst) in enumerate(pre_qk):
                nc.vector.tensor_copy(dst[:, 0:512], qk_ps[i][:])
            for tb in range(4):
                nc.vector.tensor_copy(
                    v_all[:, tb, :, 0:64],
                    v_ps[tb][:].rearrange("p (h d) -> p h d", d=HD),
                )

        with (
            tc.tile_pool(name="psS", bufs=1, space=PSUM) as psS,
            tc.tile_pool(name="psY", bufs=1, space=PSUM) as psY,
            tc.tile_pool(name="pF", bufs=2, space=PSUM) as pF,
            tc.tile_pool(name="pP", bufs=4) as pP,
            tc.tile_pool(name="pN", bufs=3) as pN,
            tc.tile_pool(name="pO", bufs=3) as pO,
        ):
            # ---- filler generators (each yield ~= 2 matmuls of PE work) ----
            def gen_qk(w_sb, dst, qc):
                ps = pF.tile([128, 512], F32, tag="fill", name="fqk")
                for ci in range(CI):
                    nc.tensor.matmul(
                        ps[:], w_sb[:, ts(ci, 128)], xT_sb[ci][:, ts(qc, 512)],
                        start=(ci == 0), stop=(ci == CI - 1),
                    )
                    if ci % 2 == 1 and ci < CI - 1:
                        yield
                nc.vector.tensor_copy(dst[:, ts(qc, 512)], ps[:])

            def gen_v(tb):
                psv = pF.tile([128, 256], F32, tag="fill", name="fv")
                for ci in range(CI):
                    nc.tensor.matmul(
                        psv[:], xT_sb[ci][:, ts(tb, 128)],
                        wv_sb[:, ts(ci, 256)],
                        start=(ci == 0), stop=(ci == CI - 1),
                    )
                    if ci % 2 == 1 and ci < CI - 1:
                        yield
                nc.vector.tensor_copy(
                    v_all[:, tb, :, 0:64],
                    psv[:].rearrange("p (h d) -> p h d", d=HD),
                )

            def gen_proj(tb, scalar_cast=False):
                ob = pO.tile([128, 1024], BF16, tag="ob", name=f"ob{tb}")
                for cc in range(2):
                    po = pF.tile([128, 512], F32, tag="fill", name="fpo")
                    for p in range(PAIRS):
                        nc.tensor.matmul(
                            po[:], yT[p][:, ts(tb, 128)],
                            wp_sb[:, p * 1024 + cc * 512:
                                  p * 1024 + cc * 512 + 512],
                            start=(p == 0), stop=(p == PAIRS - 1),
                        )
                    if scalar_cast:   # tail units: ACT is idle by then
                        nc.scalar.copy(ob[:, ts(cc, 512)], po[:])
                    else:
                        nc.vector.tensor_copy(ob[:, ts(cc, 512)], po[:])
                    yield
                nc.sync.dma_start(out_d[ts(tb, 128), :], ob[:])

            fillers = deque()        # urgent: deadline-bound qk/v chains
            fillers_lazy = deque()   # lazy: proj chunks (no deadline)
            reserve = []             # proj units held back for the tail
            active = [None]
            done = set()

            def _advance(no_lazy=False):
                """Advance the head filler generator one yield; True if work done."""
                while True:
                    if active[0] is None:
                        if fillers:
                            active[0] = fillers.popleft()
                        elif fillers_lazy and not no_lazy:
                            active[0] = fillers_lazy.popleft()
                        else:
                            return False
                    try:
                        next(active[0][1])
                        return True
                    except StopIteration:
                        done.add(active[0][0])
                        active[0] = None

            def drip(n=DRIP_N, no_lazy=False):
                for _ in range(n):
                    if not _advance(no_lazy):
                        return

            def drain(label):
                """Force-finish all fillers up to and including `label`."""
                while label not in done:
                    if not _advance():
                        return

            # ---- attention ----
            def attn_qc(p, qc):
                nkb = 4 * qc + 4
                ypt = [psY.tile([128, 512], F32, tag=f"ypt{hh}", bufs=1,
                                name=f"ypt_p{p}q{qc}h{hh}") for hh in (0, 1)]

                def emit_pv(kb0, pts):
                    for j in (0, 1):
                        kb = kb0 + j
                        col = max(0, (kb - 4 * qc) * 128)
                        for hh in (0, 1):
                            nc.tensor.matmul(
                                ypt[hh][0:65, col:512],
                                v_all[:, kb, 2 * p + hh, 0:65],
                                pts[hh][:, j * 512 + col: (j + 1) * 512],
                                start=(kb == 0), stop=(kb == nkb - 1),
                            )

                pend = None
                for kb0 in range(0, nkb, 2):
                    if p == 0 and qc > 0 and kb0 == 4 * qc:
                        drain(f"v{4 * qc + 3}")   # diag PVs need this qc's v
                    sps = [psS.tile([128, 1024], F32, tag=f"sp{hh}", bufs=1,
                                    name=f"sp{hh}") for hh in (0, 1)]
                    for hh in (0, 1):
                        off = hh * 64
                        for j in (0, 1):
                            kb = kb0 + j
                            col = max(0, (kb - 4 * qc) * 128)
                            nc.tensor.matmul(
                                sps[hh][:, j * 512 + col: (j + 1) * 512],
                                kT[p][off:off + 64, ts(kb, 128)],
                                qT[p][off:off + 64,
                                      qc * 512 + col: (qc + 1) * 512],
                                start=True, stop=True,
                            )
                    # the last (all-diagonal) step only has valid S columns
                    # from 256 on — skip the dead columns in the exp
                    e0 = 256 if kb0 == 4 * qc + 2 else 0
                    pts = []
                    for hh in (0, 1):
                        pt = pP.tile([128, 1024], BF16, tag=f"pt{hh}",
                                     name=f"pt{hh}")
                        nc.scalar.activation(pt[:, e0:1024],
                                             sps[hh][:, e0:1024], AF.Exp)
                        for j in (0, 1):
                            kb = kb0 + j
                            if kb >= 4 * qc:
                                col = j * 512 + (kb - 4 * qc) * 128
                                nc.vector.tensor_mul(
                                    pt[:, col:col + 128],
                                    pt[:, col:col + 128], mask_d[:],
                                )
                        pts.append(pt)
                    # early passes are short but chain supply is front-loaded:
                    # drip faster there (ACT is not yet the rate limiter)
                    drip(4 if qc <= 1 else DRIP_N)
                    if pend is not None:
                        emit_pv(*pend)
                    pend = (kb0, pts)
                emit_pv(*pend)
                # normalize: yT = num * (1/l); l lives in row 64 of ypt.
                # In the final pass the l-copy runs on ACT (idle by then) to
                # shorten the latency chain gating the last projections.
                for hh in (0, 1):
                    off = hh * 64
                    l_sb = pN.tile([1, 512], F32, tag="lr")
                    if p == 1 and qc == NQC - 1:
                        nc.scalar.copy(l_sb[:], ypt[hh][64:65, :])
                    else:
                        nc.vector.tensor_copy(l_sb[:], ypt[hh][64:65, :])
                    il = pN.tile([1, 512], F32, tag="il")
                    nc.vector.reciprocal_approx_fast(il[:], l_sb[:])
                    rl = pN.tile([64, 512], F32, tag="rl")
                    nc.gpsimd.partition_broadcast(rl[:], il[:])
                    nc.vector.tensor_mul(
                        yT[p][off:off + 64, ts(qc, 512)], ypt[hh][0:64, :],
                        rl[:],
                    )

            # ---- main schedule ----
            # queue order per qc: chains for qc+1 (deadline-bound, drained
            # just-in-time) then proj for qc (deadline-free; leftovers keep
            # the PE warm through the final normalize + projection tail)
            # supply shape: k0/q0 chains for qc+1 drip through iteration qc
            # (they gate the next pass); v + pair-1 chains for qc drip inside
            # iteration qc itself so the long late passes keep the PE fed
            for qc in range(NQC):
                if qc > 0:
                    for tb in range(4 * qc, 4 * qc + 4):
                        fillers.append((f"v{tb}", gen_v(tb)))
                    fillers.append((f"k1q{qc}", gen_qk(wk_sb[1], kT[1], qc)))
                    fillers.append((f"q1q{qc}", gen_qk(wq_sb[1], qT[1], qc)))
                if qc + 1 < NQC:
                    nqc = qc + 1
                    fillers.append((f"k0q{nqc}", gen_qk(wk_sb[0], kT[0], nqc)))
                    fillers.append((f"q0q{nqc}", gen_qk(wq_sb[0], qT[0], nqc)))
                attn_qc(0, qc)
                if qc > 0:
                    drain(f"q1q{qc}")
                attn_qc(1, qc)
                if qc + 1 < NQC:
                    drain(f"q0q{qc + 1}")
                for tb in range(4 * qc, 4 * qc + 4):
                    # LIFO for qc<3 so late passes drip the newest proj first;
                    # two qc2 units are held back to cover the final
                    # normalize window; qc3's own (normalize-gated) units
                    # stay at the back so the tail never stalls on them
                    if qc == NQC - 1:
                        fillers_lazy.append((f"proj{tb}", gen_proj(tb, True)))
                    elif qc == NQC - 2 and tb >= 4 * qc + 2:
                        reserve.append((f"proj{tb}", gen_proj(tb)))
                    else:
                        fillers_lazy.appendleft((f"proj{tb}", gen_proj(tb)))
            fillers_lazy.extendleft(reversed(reserve))
            while _advance():
                pass


_NC_CACHE = None


def _build():
    global _NC_CACHE
    if _NC_CACHE is not None:
        return _NC_CACHE
    nc = bacc.Bacc("TRN2", target_bir_lowering=False, debug=False,
                   num_devices=N_CORES)
    xT_d = nc.dram_tensor("xT", [CI, 128, T], BF16, kind="ExternalInput")
    w0_d = nc.dram_tensor("w0", [128, 2048], BF16, kind="ExternalInput")
    wv_d = nc.dram_tensor("wv", [128, 2048], BF16, kind="ExternalInput")
    w1_d = nc.dram_tensor("w1", [128, 2048], BF16, kind="ExternalInput")
    wp_d = nc.dram_tensor("wp", [128, 2048], BF16, kind="ExternalInput")
    out_d = nc.dram_tensor("out", [T, C], BF16, kind="ExternalOutput")

    with tile.TileContext(nc) as tc:
        _emit(tc, nc, xT_d, w0_d, wv_d, w1_d, wp_d, out_d)
    nc.compile()
    _NC_CACHE = nc
    return nc


def _pack_pair(m):
    # [1024, 128] -> lhsT chunks layout [128, 8*128]
    return np.ascontiguousarray(
        m.reshape(CI, 128, 128).transpose(1, 0, 2).reshape(128, 1024))


def _io_np(a):
    import ml_dtypes
    return np.ascontiguousarray(a.astype(ml_dtypes.bfloat16))


def _in_maps(x, w_attn, w_proj):
    x = np.asarray(x, dtype=np.float32)
    w_attn = np.asarray(w_attn, dtype=np.float32)
    w_proj = np.asarray(w_proj, dtype=np.float32)
    xT = [_io_np(np.ascontiguousarray(x[b].T).reshape(CI, 128, T))
          for b in range(B)]
    maps = []
    for core in range(N_CORES):
        b, g = core // HPC, core % HPC
        cols = slice(g * 256, (g + 1) * 256)
        wk_full = w_attn[:, 0 * C:1 * C][:, cols]
        wq_full = w_attn[:, 1 * C:2 * C][:, cols] * np.float32(1.0 / np.sqrt(HD))
        wv_full = w_attn[:, 2 * C:3 * C][:, cols]
        wq_in = [_pack_pair(wq_full[:, p * 128:(p + 1) * 128])
                 for p in range(PAIRS)]
        wk_in = [_pack_pair(wk_full[:, p * 128:(p + 1) * 128])
                 for p in range(PAIRS)]
        wv_in = wv_full.reshape(CI, 128, 256).transpose(1, 0, 2).reshape(128, 2048)
        wp_in = (w_proj[g * 256:(g + 1) * 256, :]
                 .reshape(PAIRS, 128, 1024).transpose(1, 0, 2).reshape(128, 2048))
        w0_in = np.concatenate([wk_in[0], wq_in[0]], axis=1)
        w1_in = np.concatenate([wk_in[1], wq_in[1]], axis=1)
        maps.append({"xT": xT[b], "w0": _io_np(w0_in), "wv": _io_np(wv_in),
                     "w1": _io_np(w1_in), "wp": _io_np(wp_in)})
    return maps


def _assemble(results, b_proj):
    b_proj = np.asarray(b_proj, dtype=np.float32)
    out = np.zeros((B, T, C), dtype=np.float32)
    for core in range(N_CORES):
        out[core // HPC] += np.asarray(results[core]["out"], dtype=np.float32)
    out += b_proj[None, None, :]
    return out


def kernel(x, w_attn, w_proj, b_proj):
    nc = _build()
    maps = _in_maps(x, w_attn, w_proj)
    res = run_bass_kernel_spmd(nc, maps, list(range(N_CORES)))
    return _assemble(res.results, b_proj)


def kernel_traced(x, w_attn, w_proj, b_proj):
    """Like kernel() but with NTFF tracing; returns (out, BassKernelResults)."""
    nc = _build()
    maps = _in_maps(x, w_attn, w_proj)
    res = run_bass_kernel_spmd(nc, maps, list(range(N_CORES)), trace=True)
    return _assemble(res.results, b_proj), res
